# revision 25
# baseline (speedup 1.0000x reference)
"""Trainium2 (Bass/Tile) multi-head attention across 8 NeuronCores.

Problem: MHA with B=2, T=2048, D=1024, 16 heads (head_dim 64), causal +
key-padding mask, fp32 in/out.

Sharding: head-parallel attention. Core c owns heads {2c, 2c+1} for both
batches: column-parallel Q/K/V projections (its 128 of 1024 feature dims),
per-head causal attention kept device-local, then an AllToAll re-shards the
normalized ctx^T from head-split to sequence-split so each core finishes
its 512 rows of the output projection locally (full Wo, bias added once).

Layout/precision choices:
- All matmul operands are bf16 (PSUM accumulation stays f32): same PE
  cycle count as float32r at n>=256, but half the SBUF/HBM/ldweights
  traffic on a utilization-throttled part.
- x^T streamed per 512-row chunk as 8 parallel per-dc DMAs (the DMA
  engines run ~18GB/s per queue; one big DMA would serialize).
- V^T transposed on PE to [t, d] blocks in ONE [128,128] transpose per
  t-block (both heads), stored as [d0|ones|d1] so head0's ctx matmul
  reads cols [0,65) (denominator lands on psum row 64) and head1 reads
  cols [64,129) reversed (denominator on psum row 0). The ones column
  makes attn @ V accumulate the softmax denominator for free.
- Scores are computed transposed (S^T[k, q]) and exponentiated without a
  running max (inputs are scaled so |scores| < ~4; softmax is shift-
  invariant, exp cannot overflow). For the 4 diagonal k-blocks of each
  q-chunk, the scores/exp/ctx ops are narrowed to the columns that are
  not fully masked, and only the 128-wide partial band is multiplied by
  a triangle mask (on GpSimd) after exp, which is exact.
- Softmax normalize off the PE: reciprocal_approx_fast on the denominator
  row, GpSimd partition-broadcast, one DVE multiply.
- ctx^T normalized into bf16; AllToAll payload is bf16 (1MB/core).
"""

import sys

for _p in ("/opt/trn_rl_repo", "/root/.axon_site/_ro/trn_rl_repo"):
    if _p not in sys.path:
        sys.path.insert(0, _p)

import numpy as np
import ml_dtypes

import concourse.bass as bass
import concourse.bacc as bacc
import concourse.mybir as mybir
import concourse.tile as tile
from concourse.bass_utils import run_bass_kernel_spmd
from concourse.vector_clock import ScopedClock

F32 = mybir.dt.float32
F32R = mybir.dt.float32r
BF16 = mybir.dt.bfloat16

N_CORES = 8
B, T, D = 2, 2048, 1024
H, HD = 16, 64
TT = B * T              # 4096 flat rows
QC = 512                # q-chunk (columns per S^T tile)
KB = 128                # k-block (partitions per S^T tile)
NQC = T // QC           # 4 q-chunks per batch
NTB = T // KB           # 16 t-blocks per batch
DC = D // 128           # 8 contraction chunks
VW = 2 * HD + 2         # 130: [d0 | ones | d1 | ones] per t-block


class _SplitDrainTileContext(tile.TileContext):
    """TileContext whose kernel-tail drain splits its semaphore waits.

    The walrus build here rejects >1 sync-wait on a CTRL_NO instruction
    ("Too many sync wait commands"). Stock TileContext attaches every
    engine/queue's final clock wait to the single kernel-tail Drain. A
    probe NoOp discovers the waits (and advances the elision state); we
    emit one single-wait instruction per semaphore, then a bare Drain.
    """

    def _drain_and_barrier(self, tick_clock, wait_clock):
        probe = mybir.InstNoOp(
            name=f"I-drainprobe-{self.nc.next_id()}", ins=[], outs=[]
        )
        probe.engine = mybir.EngineType.SP
        wait_clock.add_sem_waits(
            probe, ScopedClock({None: tick_clock.global_clock})
        )
        waits = list(probe.sync_info.on_wait) if probe.sync_info else []
        by_name = {h.name: h for h in self.sems.allocated().values()}
        for w in waits:
            self.nc.sync.wait_ge(by_name[w.ant_name], w.wait_value)
        self.nc.sync.drain()

        self.nc.all_engine_barrier()
        popped = self.nc._tile_sem_poison_stack.pop()
        assert popped is self._sem_poison
        self.nc.clear_and_free_semaphores(list(self.sems.allocated().values()))
        self.nc.all_engine_barrier()


def _build(with_padding: bool):
    nc = bacc.Bacc(
        trn_type="TRN2",
        target_bir_lowering=False,
        debug=False,
        num_devices=N_CORES,
    )

    xT_e = nc.declare_dram_parameter("xT", [B * NQC, DC, 128, QC], BF16, isOutput=False)
    wq_e = nc.declare_dram_parameter("wq", [128, DC * 128], BF16, isOutput=False)
    wk_e = nc.declare_dram_parameter("wk", [128, DC * 128], BF16, isOutput=False)
    wv_e = nc.declare_dram_parameter("wv", [128, DC * 128], BF16, isOutput=False)
    wo_e = nc.declare_dram_parameter("wo", [128, DC * D], BF16, isOutput=False)
    bo_e = nc.declare_dram_parameter("bo_row", [1, D], F32, isOutput=False)
    tri_e = nc.declare_dram_parameter("tri", [128, 128], BF16, isOutput=False)
    idn_e = nc.declare_dram_parameter("ident", [128, 128], F32, isOutput=False)
    one_e = nc.declare_dram_parameter("ones64", [1, HD], F32R, isOutput=False)
    if with_padding:
        # 1.0 = valid key, 0.0 = padded; [b, kb, 128, 1]
        pad_e = nc.declare_dram_parameter(
            "padcol", [B, NTB, 128, 1], F32, isOutput=False
        )
    out_e = nc.declare_dram_parameter("out", [TT // N_CORES, D], F32, isOutput=True)

    with _SplitDrainTileContext(nc) as tc:
        cst = tc.alloc_tile_pool(name="cst", bufs=1)
        per = tc.alloc_tile_pool(name="per", bufs=1)

        # Critical-path DMAs first: projection weights, then small consts.
        wq_sb = cst.tile([128, DC * 128], BF16)
        wk_sb = cst.tile([128, DC * 128], BF16)
        wv_sb = cst.tile([128, DC * 128], BF16)
        tri_sb = cst.tile([128, 128], BF16)
        idn_sb = cst.tile([128, 128], F32)
        one_sb = cst.tile([1, HD], F32R)
        bo_sb = cst.tile([1, D], F32)
        nc.sync.dma_start(wq_sb[:], wq_e[:])
        xt0 = cst.tile([128, DC * QC], BF16)
        for dc in range(DC):
            nc.sync.dma_start(xt0[:, dc * QC:(dc + 1) * QC], xT_e[0, dc])
        nc.sync.dma_start(wk_sb[:], wk_e[:])
        nc.sync.dma_start(wv_sb[:], wv_e[:])
        nc.sync.dma_start(tri_sb[:], tri_e[:])
        nc.sync.dma_start(idn_sb[:], idn_e[:])
        nc.sync.dma_start(one_sb[:], one_e[:])
        nc.sync.dma_start(bo_sb[:], bo_e[:])
        if with_padding:
            pad_sb = cst.tile([128, B * NTB], F32)
            for b in range(B):
                for tb in range(NTB):
                    nc.sync.dma_start(
                        pad_sb[:, b * NTB + tb: b * NTB + tb + 1], pad_e[b, tb]
                    )

        # Persistent per-batch tensors: dims on partitions (2 heads x 64).
        qt = [per.tile([128, T], BF16, name=f"qt{b}") for b in range(B)]
        kt = [per.tile([128, T], BF16, name=f"kt{b}") for b in range(B)]
        # V blocks [t, d]: per (b, tb): [d0(64) | ones | d1(64)] cols.
        vx = [per.tile([128, NTB * VW], BF16, name=f"vx{b}") for b in range(B)]
        ctxT = per.tile([128, TT], BF16)
        ctxf = [per.tile([128, N_CORES * 128], BF16, name=f"cf{rb}")
                for rb in range(4)]
        wo_sb = per.tile([128, DC * D], BF16)
        bo_bc = per.tile([128, D], F32)

        # ---- Phase A: projections (+ V transposes interleaved) ----
        with (
            tc.tile_pool(name="xtp", bufs=3) as xtp,
            tc.tile_pool(name="vtp", bufs=1) as vtp,
            tc.tile_pool(name="psA", bufs=3, space="PSUM") as psA,
            tc.tile_pool(name="psT", bufs=2, space="PSUM") as psT,
        ):
            vt = [vtp.tile([128, T], F32, name=f"vt{b}") for b in range(B)]
            for b in range(B):
                vxv = vx[b].rearrange("p (t c) -> p t c", c=VW)
                nc.vector.memset(vxv[:, :, HD], 1.0)
                nc.vector.memset(vxv[:, :, 2 * HD + 1], 1.0)
            for b in range(B):
                for tci in range(NQC):
                    g = NQC * b + tci
                    if g == 0:
                        xt = xt0
                    else:
                        xt = xtp.tile([128, DC * QC], BF16)
                        for dc in range(DC):
                            nc.sync.dma_start(
                                xt[:, dc * QC:(dc + 1) * QC], xT_e[g, dc]
                            )
                    for w_sb, dst in (
                        (wq_sb, qt[b]),
                        (wk_sb, kt[b]),
                        (wv_sb, vt[b]),
                    ):
                        ps = psA.tile([128, QC], F32)
                        for dc in range(DC):
                            nc.tensor.matmul(
                                ps[:],
                                w_sb[:, dc * 128:(dc + 1) * 128],
                                xt[:, dc * QC:(dc + 1) * QC],
                                start=(dc == 0),
                                stop=(dc == DC - 1),
                            )
                        nc.vector.tensor_copy(
                            dst[:, tci * QC:(tci + 1) * QC], ps[:]
                        )
                    # Transpose this chunk's 4 V t-blocks: [dims,t]->[t,dims].
                    for tb in range(tci * 4, tci * 4 + 4):
                        tp = psT.tile([128, 128], F32)
                        nc.tensor.transpose(
                            tp[:],
                            vt[b][:, tb * 128:(tb + 1) * 128],
                            idn_sb[:],
                        )
                        nc.vector.tensor_copy(
                            vx[b][:, tb * VW: tb * VW + HD], tp[:, 0:HD]
                        )
                        nc.vector.tensor_copy(
                            vx[b][:, tb * VW + HD + 1: tb * VW + 2 * HD + 1],
                            tp[:, HD:2 * HD],
                        )

        # Deferred big loads: needed only by Phase D.
        nc.sync.dma_start(wo_sb[:], wo_e[:])
        nc.gpsimd.partition_broadcast(bo_bc[:], bo_sb[:], channels=128)

        # ---- Phase B: attention (qc-outer so A2A chunks finish early) ----
        with tc.tile_pool(name="dramp", bufs=1, space="DRAM") as dramp:
          send = [dramp.tile([N_CORES, 128, 128], BF16, name=f"send{rb}")
                  for rb in range(4)]
          recv = [dramp.tile([N_CORES, 128, 128], BF16, name=f"recv{rb}")
                  for rb in range(4)]
          with (
            tc.tile_pool(name="psS", bufs=4, space="PSUM") as psS,
            tc.tile_pool(name="psC", bufs=2, space="PSUM") as psC,
            tc.tile_pool(name="psB", bufs=2, space="PSUM") as psB,
            tc.tile_pool(name="pP", bufs=8) as pP,
            tc.tile_pool(name="pL", bufs=4) as pL,
          ):
            def emit_normalize(st):
                b_, qc_, hh_, cps_ = st
                hs_ = slice(hh_ * HD, (hh_ + 1) * HD)
                lrow = pL.tile([1, QC], F32R)
                nc.vector.tensor_copy(lrow[:], cps_[HD:HD + 1, :])
                bps = psB.tile([HD, QC], F32)
                nc.tensor.matmul(
                    bps[:], one_sb[0:1, :HD], lrow[:],
                    start=True, stop=True,
                )
                rbr = pL.tile([HD, QC], F32)
                nc.vector.reciprocal_approx_fast(rbr[:], bps[:])
                nc.vector.tensor_mul(
                    ctxT[hs_, b_ * T + qc_ * QC: b_ * T + (qc_ + 1) * QC],
                    cps_[0:HD, :],
                    rbr[:],
                )
                if hh_ == 1:
                    # Chunk (b_, qc_) fully normalized: stage its 4 sends
                    # into collective rb=(b_, qc_//2), slots (qc_%2)*4+0..3.
                    rb_ = 2 * b_ + qc_ // 2
                    for jj in range(4):
                        j = (qc_ % 2) * 4 + jj
                        col = b_ * T + (qc_ // 2) * 1024 + j * 128
                        nc.sync.dma_start(
                            send[rb_][j], ctxT[:, col: col + 128]
                        )
                    if qc_ % 2 == 1:
                        nc.gpsimd.collective_compute(
                            "AllToAll",
                            mybir.AluOpType.bypass,
                            replica_groups=[list(range(N_CORES))],
                            ins=[send[rb_].opt()],
                            outs=[recv[rb_].opt()],
                        )

            pending = None
            for b in range(B):
                for qc in range(NQC):
                    nkb = (T // KB // NQC) * (qc + 1)
                    for hh in range(2):
                        hs = slice(hh * HD, (hh + 1) * HD)
                        # head hh: V cols [hh*65, hh*65+65); denom row 64.
                        vc0 = hh * (HD + 1)
                        cps = psC.tile([HD + 1, QC], F32)
                        for kb in range(nkb):
                            j = kb - 4 * qc
                            c0 = j * 128 if j > 0 else 0
                            sps = psS.tile([128, QC], F32)
                            nc.tensor.matmul(
                                sps[:, c0:],
                                kt[b][hs, kb * KB:(kb + 1) * KB],
                                qt[b][hs, qc * QC + c0:(qc + 1) * QC],
                                start=True,
                                stop=True,
                            )
                            pt = pP.tile([128, QC], BF16)
                            nc.scalar.activation(
                                pt[:, c0:], sps[:, c0:],
                                mybir.ActivationFunctionType.Exp,
                            )
                            if j >= 0:
                                nc.gpsimd.tensor_mul(
                                    pt[:, j * 128:(j + 1) * 128],
                                    pt[:, j * 128:(j + 1) * 128],
                                    tri_sb[:],
                                )
                            if with_padding:
                                nc.vector.tensor_scalar_mul(
                                    pt[:, c0:],
                                    pt[:, c0:],
                                    pad_sb[:, b * NTB + kb: b * NTB + kb + 1],
                                )
                            nc.tensor.matmul(
                                cps[:, c0:],
                                vx[b][:, kb * VW + vc0: kb * VW + vc0 + HD + 1],
                                pt[:, c0:],
                                start=(kb == 0),
                                stop=(kb == nkb - 1),
                                skip_group_check=True,
                            )
                        # Normalize the PREVIOUS chunk now: its denominator
                        # row has been ready for a whole chunk, so the PE
                        # broadcast doesn't bubble waiting on the DVE copy.
                        if pending is not None:
                            emit_normalize(pending)
                        pending = (b, qc, hh, cps)
            emit_normalize(pending)

          # Recv loads on the sync queue (idle after sends) so their
          # collective waits don't convoy any compute engine.
          for rb in range(4):
              for i in range(N_CORES):
                  nc.sync.dma_start(
                      ctxf[rb][:, i * 128:(i + 1) * 128], recv[rb][i]
                  )

          # ---- Phase D: out-proj; row block rb = (b, qh) of my 128 rows ----
          with (
              tc.tile_pool(name="psO", bufs=2, space="PSUM") as psO,
              tc.tile_pool(name="pO", bufs=2) as pO,
          ):
              for rb in range(4):
                  ob = pO.tile([128, D], F32)
                  for jc in range(2):
                      ops = psO.tile([128, 512], F32)
                      for dc in range(DC):
                          nc.tensor.matmul(
                              ops[:],
                              ctxf[rb][:, dc * 128:(dc + 1) * 128],
                              wo_sb[:, dc * D + jc * 512:
                                    dc * D + jc * 512 + 512],
                              start=(dc == 0),
                              stop=(dc == DC - 1),
                          )
                      nc.vector.scalar_tensor_tensor(
                          ob[:, jc * 512:(jc + 1) * 512],
                          ops[:],
                          1.0,
                          bo_bc[:, jc * 512:(jc + 1) * 512],
                          op0=mybir.AluOpType.mult,
                          op1=mybir.AluOpType.add,
                      )
                      nc.sync.dma_start(
                          out_e[rb * 128:(rb + 1) * 128,
                                jc * 512:(jc + 1) * 512],
                          ob[:, jc * 512:(jc + 1) * 512],
                      )
        per.release()
        cst.release()

    nc.finalize()
    return nc


_CACHE = {}


def _get_nc(with_padding: bool):
    if with_padding not in _CACHE:
        _CACHE[with_padding] = _build(with_padding)
    return _CACHE[with_padding]


def _prepare_in_maps(x, Wq, Wk, Wv, Wo, bo, key_padding_mask):
    bf = ml_dtypes.bfloat16
    x = np.asarray(x, dtype=np.float32)
    Wq = np.asarray(Wq, dtype=np.float32)
    Wk = np.asarray(Wk, dtype=np.float32)
    Wv = np.asarray(Wv, dtype=np.float32)
    Wo = np.asarray(Wo, dtype=np.float32)
    bo = np.asarray(bo, dtype=np.float32)
    pad = np.asarray(key_padding_mask)

    with_padding = bool(pad.any())

    # [g, dc, p, t]: one 128KB DMA per (chunk, d-chunk).
    xT = np.ascontiguousarray(
        x.reshape(B * NQC, QC, DC, 128).transpose(0, 2, 3, 1)
    ).astype(bf)
    # Fold the 1/sqrt(head_dim) score scale into Wq (power of two: exact).
    Wq_s = Wq * np.float32(1.0 / np.sqrt(HD))

    # tri[k, c] = 1.0 where k <= c (keep), 0 above-diagonal k > c.
    tri = (np.arange(128)[:, None] <= np.arange(128)[None, :]).astype(bf)
    ident = np.eye(128, dtype=np.float32)
    ones64 = np.ones((1, HD), dtype=np.float32)
    bo_row = np.ascontiguousarray(bo.reshape(1, D))

    def wsb(W, cols):
        # SBUF layout [128, DC*128]: [p, dc*128 + c] = W[dc*128 + p, cols[c]]
        blk = W[:, cols].reshape(DC, 128, 128)
        return np.ascontiguousarray(
            blk.transpose(1, 0, 2).reshape(128, DC * 128)
        ).astype(bf)

    wo3 = np.ascontiguousarray(
        Wo.reshape(DC, 128, D).transpose(1, 0, 2).reshape(128, DC * D)
    ).astype(bf)

    in_maps = []
    for c in range(N_CORES):
        cols = slice(c * 128, (c + 1) * 128)
        m = {
            "xT": xT,
            "wq": wsb(Wq_s, cols),
            "wk": wsb(Wk, cols),
            "wv": wsb(Wv, cols),
            "wo": wo3,
            "bo_row": bo_row,
            "tri": tri,
            "ident": ident,
            "ones64": ones64,
        }
        if with_padding:
            m["padcol"] = np.ascontiguousarray(
                (~pad).astype(np.float32).reshape(B, NTB, 128, 1)
            )
        in_maps.append(m)
    return with_padding, in_maps


def _run(with_padding, in_maps, trace=False):
    nc = _get_nc(with_padding)
    return run_bass_kernel_spmd(
        nc, in_maps, core_ids=list(range(N_CORES)), trace=trace
    )


def kernel(x, Wq, Wk, Wv, Wo, bo, key_padding_mask):
    with_padding, in_maps = _prepare_in_maps(
        x, Wq, Wk, Wv, Wo, bo, key_padding_mask
    )
    res = _run(with_padding, in_maps)
    # Core c's out row-block rb=2b+qh covers rows [qh*1024 + c*128, +128).
    full = np.empty((B, T, D), dtype=np.float32)
    for c in range(N_CORES):
        o = res.results[c]["out"]
        for b in range(B):
            for qh in range(2):
                r0 = qh * 1024 + c * 128
                full[b, r0:r0 + 128] = o[(2 * b + qh) * 128:
                                         (2 * b + qh + 1) * 128]
    return full


# revision 26
# speedup vs baseline: 1.0798x; 1.0798x over previous
"""Trainium2 (Bass/Tile) multi-head attention across 8 NeuronCores.

Problem: MHA with B=2, T=2048, D=1024, 16 heads (head_dim 64), causal +
key-padding mask, fp32 in/out.

Sharding: head-parallel attention. Core c owns heads {2c, 2c+1} for both
batches: column-parallel Q/K/V projections (its 128 of 1024 feature dims),
per-head causal attention kept device-local, then an AllToAll re-shards the
normalized ctx^T from head-split to sequence-split so each core finishes
its 512 rows of the output projection locally (full Wo, bias added once).

Layout/precision choices:
- All matmul operands are bf16 (PSUM accumulation stays f32): same PE
  cycle count as float32r at n>=256, but half the SBUF/HBM/ldweights
  traffic on a utilization-throttled part.
- x^T streamed per 512-row chunk as 8 parallel per-dc DMAs (the DMA
  engines run ~18GB/s per queue; one big DMA would serialize).
- V^T transposed on PE to [t, d] blocks in ONE [128,128] transpose per
  t-block (both heads), stored as [d0|ones|d1] so head0's ctx matmul
  reads cols [0,65) (denominator lands on psum row 64) and head1 reads
  cols [64,129) reversed (denominator on psum row 0). The ones column
  makes attn @ V accumulate the softmax denominator for free.
- Scores are computed transposed (S^T[k, q]) and exponentiated without a
  running max (inputs are scaled so |scores| < ~4; softmax is shift-
  invariant, exp cannot overflow). For the 4 diagonal k-blocks of each
  q-chunk, the scores/exp/ctx ops are narrowed to the columns that are
  not fully masked, and only the 128-wide partial band is multiplied by
  a triangle mask (on GpSimd) after exp, which is exact.
- Softmax normalize off the PE: reciprocal_approx_fast on the denominator
  row, GpSimd partition-broadcast, one DVE multiply.
- ctx^T normalized into bf16; AllToAll payload is bf16 (1MB/core).
"""

import sys

for _p in ("/opt/trn_rl_repo", "/root/.axon_site/_ro/trn_rl_repo"):
    if _p not in sys.path:
        sys.path.insert(0, _p)

import numpy as np
import ml_dtypes

import concourse.bass as bass
import concourse.bacc as bacc
import concourse.mybir as mybir
import concourse.tile as tile
from concourse.bass_utils import run_bass_kernel_spmd
from concourse.vector_clock import ScopedClock

F32 = mybir.dt.float32
F32R = mybir.dt.float32r
BF16 = mybir.dt.bfloat16

N_CORES = 8
B, T, D = 2, 2048, 1024
H, HD = 16, 64
TT = B * T              # 4096 flat rows
QC = 512                # q-chunk (columns per S^T tile)
KB = 128                # k-block (partitions per S^T tile)
NQC = T // QC           # 4 q-chunks per batch
NTB = T // KB           # 16 t-blocks per batch
DC = D // 128           # 8 contraction chunks
VW = 2 * HD + 2         # 130: [d0 | ones | d1 | ones] per t-block


class _SplitDrainTileContext(tile.TileContext):
    """TileContext whose kernel-tail drain splits its semaphore waits.

    The walrus build here rejects >1 sync-wait on a CTRL_NO instruction
    ("Too many sync wait commands"). Stock TileContext attaches every
    engine/queue's final clock wait to the single kernel-tail Drain. A
    probe NoOp discovers the waits (and advances the elision state); we
    emit one single-wait instruction per semaphore, then a bare Drain.
    """

    def _drain_and_barrier(self, tick_clock, wait_clock):
        probe = mybir.InstNoOp(
            name=f"I-drainprobe-{self.nc.next_id()}", ins=[], outs=[]
        )
        probe.engine = mybir.EngineType.SP
        wait_clock.add_sem_waits(
            probe, ScopedClock({None: tick_clock.global_clock})
        )
        waits = list(probe.sync_info.on_wait) if probe.sync_info else []
        by_name = {h.name: h for h in self.sems.allocated().values()}
        for w in waits:
            self.nc.sync.wait_ge(by_name[w.ant_name], w.wait_value)
        self.nc.sync.drain()

        self.nc.all_engine_barrier()
        popped = self.nc._tile_sem_poison_stack.pop()
        assert popped is self._sem_poison
        self.nc.clear_and_free_semaphores(list(self.sems.allocated().values()))
        self.nc.all_engine_barrier()


def _build(with_padding: bool):
    nc = bacc.Bacc(
        trn_type="TRN2",
        target_bir_lowering=False,
        debug=False,
        num_devices=N_CORES,
    )

    xT_e = nc.declare_dram_parameter("xT", [B * NQC, DC, 128, QC], BF16, isOutput=False)
    wq_e = nc.declare_dram_parameter("wq", [128, DC * 128], BF16, isOutput=False)
    wk_e = nc.declare_dram_parameter("wk", [128, DC * 128], BF16, isOutput=False)
    wv_e = nc.declare_dram_parameter("wv", [128, DC * 128], BF16, isOutput=False)
    wo_e = nc.declare_dram_parameter("wo", [128, DC * D], BF16, isOutput=False)
    bo_e = nc.declare_dram_parameter("bo_row", [1, D], F32, isOutput=False)
    tri_e = nc.declare_dram_parameter("tri", [128, 128], BF16, isOutput=False)
    idn_e = nc.declare_dram_parameter("ident", [128, 128], F32, isOutput=False)
    one_e = nc.declare_dram_parameter("ones64", [1, HD], F32R, isOutput=False)
    if with_padding:
        # 1.0 = valid key, 0.0 = padded; [b, kb, 128, 1]
        pad_e = nc.declare_dram_parameter(
            "padcol", [B, NTB, 128, 1], F32, isOutput=False
        )
    out_e = nc.declare_dram_parameter("out", [TT // N_CORES, D], F32, isOutput=True)

    with _SplitDrainTileContext(nc) as tc:
        cst = tc.alloc_tile_pool(name="cst", bufs=1)
        per = tc.alloc_tile_pool(name="per", bufs=1)

        # Critical-path DMAs first: projection weights, then small consts.
        wq_sb = cst.tile([128, DC * 128], BF16)
        wk_sb = cst.tile([128, DC * 128], BF16)
        wv_sb = cst.tile([128, DC * 128], BF16)
        tri_sb = cst.tile([128, 128], BF16)
        idn_sb = cst.tile([128, 128], F32)
        one_sb = cst.tile([1, HD], F32R)
        bo_sb = cst.tile([1, D], F32)
        nc.sync.dma_start(wq_sb[:], wq_e[:])
        xt0 = cst.tile([128, DC * QC], BF16)
        for dc in range(DC):
            nc.sync.dma_start(xt0[:, dc * QC:(dc + 1) * QC], xT_e[0, dc])
        nc.sync.dma_start(wk_sb[:], wk_e[:])
        nc.sync.dma_start(wv_sb[:], wv_e[:])
        nc.sync.dma_start(tri_sb[:], tri_e[:])
        nc.sync.dma_start(idn_sb[:], idn_e[:])
        nc.sync.dma_start(one_sb[:], one_e[:])
        nc.sync.dma_start(bo_sb[:], bo_e[:])
        if with_padding:
            pad_sb = cst.tile([128, B * NTB], F32)
            for b in range(B):
                for tb in range(NTB):
                    nc.sync.dma_start(
                        pad_sb[:, b * NTB + tb: b * NTB + tb + 1], pad_e[b, tb]
                    )

        # Persistent per-batch tensors: dims on partitions (2 heads x 64).
        qt = [per.tile([128, T], BF16, name=f"qt{b}") for b in range(B)]
        kt = [per.tile([128, T], BF16, name=f"kt{b}") for b in range(B)]
        # V blocks [t, d]: per (b, tb): [d0(64) | ones | d1(64)] cols.
        vx = [per.tile([128, NTB * VW], BF16, name=f"vx{b}") for b in range(B)]
        ctxT = per.tile([128, TT], BF16)
        ctxf = [per.tile([128, N_CORES * 128], BF16, name=f"cf{rb}")
                for rb in range(4)]
        wo_sb = per.tile([128, DC * D], BF16)
        bo_bc = per.tile([128, D], F32)

        # ---- Phase A: projections (+ V transposes interleaved) ----
        with (
            tc.tile_pool(name="xtp", bufs=3) as xtp,
            tc.tile_pool(name="vtp", bufs=1) as vtp,
            tc.tile_pool(name="psA", bufs=3, space="PSUM") as psA,
            tc.tile_pool(name="psT", bufs=2, space="PSUM") as psT,
        ):
            vt = [vtp.tile([128, T], F32, name=f"vt{b}") for b in range(B)]
            for b in range(B):
                vxv = vx[b].rearrange("p (t c) -> p t c", c=VW)
                nc.vector.memset(vxv[:, :, HD], 1.0)
                nc.vector.memset(vxv[:, :, 2 * HD + 1], 1.0)
            for b in range(B):
                for tci in range(NQC):
                    g = NQC * b + tci
                    if g == 0:
                        xt = xt0
                    else:
                        xt = xtp.tile([128, DC * QC], BF16)
                        for dc in range(DC):
                            nc.sync.dma_start(
                                xt[:, dc * QC:(dc + 1) * QC], xT_e[g, dc]
                            )
                    for w_sb, dst in (
                        (wq_sb, qt[b]),
                        (wk_sb, kt[b]),
                        (wv_sb, vt[b]),
                    ):
                        ps = psA.tile([128, QC], F32)
                        for dc in range(DC):
                            nc.tensor.matmul(
                                ps[:],
                                w_sb[:, dc * 128:(dc + 1) * 128],
                                xt[:, dc * QC:(dc + 1) * QC],
                                start=(dc == 0),
                                stop=(dc == DC - 1),
                            )
                        nc.vector.tensor_copy(
                            dst[:, tci * QC:(tci + 1) * QC], ps[:]
                        )
                    # Transpose this chunk's 4 V t-blocks: [dims,t]->[t,dims].
                    for tb in range(tci * 4, tci * 4 + 4):
                        tp = psT.tile([128, 128], F32)
                        nc.tensor.transpose(
                            tp[:],
                            vt[b][:, tb * 128:(tb + 1) * 128],
                            idn_sb[:],
                        )
                        nc.vector.tensor_copy(
                            vx[b][:, tb * VW: tb * VW + HD], tp[:, 0:HD]
                        )
                        nc.vector.tensor_copy(
                            vx[b][:, tb * VW + HD + 1: tb * VW + 2 * HD + 1],
                            tp[:, HD:2 * HD],
                        )

        # Deferred big loads: needed only by Phase D.
        nc.sync.dma_start(wo_sb[:], wo_e[:])
        nc.gpsimd.partition_broadcast(bo_bc[:], bo_sb[:], channels=128)

        # ---- Phase B: attention (qc-outer so A2A chunks finish early) ----
        with tc.tile_pool(name="dramp", bufs=1, space="DRAM") as dramp:
          send = [dramp.tile([N_CORES, 128, 128], BF16, name=f"send{rb}")
                  for rb in range(4)]
          recv = [dramp.tile([N_CORES, 128, 128], BF16, name=f"recv{rb}")
                  for rb in range(4)]
          with (
            tc.tile_pool(name="psS", bufs=2, space="PSUM") as psS,
            tc.tile_pool(name="psC", bufs=2, space="PSUM") as psC,
            tc.tile_pool(name="psB", bufs=2, space="PSUM") as psB,
            tc.tile_pool(name="pP", bufs=8) as pP,
            tc.tile_pool(name="pL", bufs=4) as pL,
          ):
            def emit_normalize(st):
                b_, qc_, hh_, cps_ = st
                hs_ = slice(hh_ * HD, (hh_ + 1) * HD)
                lrow = pL.tile([1, QC], F32R)
                nc.vector.tensor_copy(lrow[:], cps_[HD:HD + 1, :])
                bps = psB.tile([HD, QC], F32)
                nc.tensor.matmul(
                    bps[:], one_sb[0:1, :HD], lrow[:],
                    start=True, stop=True,
                )
                rbr = pL.tile([HD, QC], F32)
                nc.vector.reciprocal_approx_fast(rbr[:], bps[:])
                nc.vector.tensor_mul(
                    ctxT[hs_, b_ * T + qc_ * QC: b_ * T + (qc_ + 1) * QC],
                    cps_[0:HD, :],
                    rbr[:],
                )
                if hh_ == 1:
                    # Chunk (b_, qc_) fully normalized: stage its 4 sends
                    # into collective rb=(b_, qc_//2), slots (qc_%2)*4+0..3.
                    rb_ = 2 * b_ + qc_ // 2
                    for jj in range(4):
                        j = (qc_ % 2) * 4 + jj
                        col = b_ * T + (qc_ // 2) * 1024 + j * 128
                        nc.sync.dma_start(
                            send[rb_][j], ctxT[:, col: col + 128]
                        )
                    if qc_ % 2 == 1:
                        nc.gpsimd.collective_compute(
                            "AllToAll",
                            mybir.AluOpType.bypass,
                            replica_groups=[list(range(N_CORES))],
                            ins=[send[rb_].opt()],
                            outs=[recv[rb_].opt()],
                        )

            pending = None
            for b in range(B):
                for qc in range(NQC):
                    nkb = (T // KB // NQC) * (qc + 1)
                    for hh in range(2):
                        hs = slice(hh * HD, (hh + 1) * HD)
                        # head hh: V cols [hh*65, hh*65+65); denom row 64.
                        vc0 = hh * (HD + 1)
                        cps = psC.tile([HD + 1, QC], F32)
                        for pb in range(nkb // 2):
                            kbA, kbB = 2 * pb, 2 * pb + 1
                            jA, jB = kbA - 4 * qc, kbB - 4 * qc
                            c0A = jA * 128 if jA > 0 else 0
                            c0B = jB * 128 if jB > 0 else 0
                            sps = psS.tile([128, 2 * QC], F32)
                            nc.tensor.matmul(
                                sps[:, c0A:QC],
                                kt[b][hs, kbA * KB:(kbA + 1) * KB],
                                qt[b][hs, qc * QC + c0A:(qc + 1) * QC],
                                start=True,
                                stop=True,
                            )
                            nc.tensor.matmul(
                                sps[:, QC + c0B:2 * QC],
                                kt[b][hs, kbB * KB:(kbB + 1) * KB],
                                qt[b][hs, qc * QC + c0B:(qc + 1) * QC],
                                start=True,
                                stop=True,
                            )
                            pt = pP.tile([128, 2 * QC], BF16)
                            if jA < 0 and jB < 0:
                                # Both halves full-width: one exp spans the
                                # two PSUM banks back-to-back.
                                nc.scalar.activation(
                                    pt[:], sps[:],
                                    mybir.ActivationFunctionType.Exp,
                                )
                            else:
                                nc.scalar.activation(
                                    pt[:, c0A:QC], sps[:, c0A:QC],
                                    mybir.ActivationFunctionType.Exp,
                                )
                                nc.scalar.activation(
                                    pt[:, QC + c0B:], sps[:, QC + c0B:],
                                    mybir.ActivationFunctionType.Exp,
                                )
                            for kb, j, c0, off in (
                                (kbA, jA, c0A, 0), (kbB, jB, c0B, QC),
                            ):
                                if j >= 0:
                                    nc.gpsimd.tensor_mul(
                                        pt[:, off + j * 128:off + (j + 1) * 128],
                                        pt[:, off + j * 128:off + (j + 1) * 128],
                                        tri_sb[:],
                                    )
                                if with_padding:
                                    nc.vector.tensor_scalar_mul(
                                        pt[:, off + c0:off + QC],
                                        pt[:, off + c0:off + QC],
                                        pad_sb[:, b * NTB + kb: b * NTB + kb + 1],
                                    )
                                nc.tensor.matmul(
                                    cps[:, c0:],
                                    vx[b][:, kb * VW + vc0: kb * VW + vc0 + HD + 1],
                                    pt[:, off + c0:off + QC],
                                    start=(kb == 0),
                                    stop=(kb == nkb - 1),
                                    skip_group_check=True,
                                )
                        # Normalize the PREVIOUS chunk now: its denominator
                        # row has been ready for a whole chunk, so the PE
                        # broadcast doesn't bubble waiting on the DVE copy.
                        if pending is not None:
                            emit_normalize(pending)
                        pending = (b, qc, hh, cps)
            emit_normalize(pending)

          # Recv loads on the sync queue (idle after sends) so their
          # collective waits don't convoy any compute engine.
          for rb in range(4):
              for i in range(N_CORES):
                  nc.sync.dma_start(
                      ctxf[rb][:, i * 128:(i + 1) * 128], recv[rb][i]
                  )

          # ---- Phase D: out-proj; row block rb = (b, qh) of my 128 rows ----
          with (
              tc.tile_pool(name="psO", bufs=2, space="PSUM") as psO,
              tc.tile_pool(name="pO", bufs=2) as pO,
          ):
              for rb in range(4):
                  ob = pO.tile([128, D], F32)
                  for jc in range(2):
                      ops = psO.tile([128, 512], F32)
                      for dc in range(DC):
                          nc.tensor.matmul(
                              ops[:],
                              ctxf[rb][:, dc * 128:(dc + 1) * 128],
                              wo_sb[:, dc * D + jc * 512:
                                    dc * D + jc * 512 + 512],
                              start=(dc == 0),
                              stop=(dc == DC - 1),
                          )
                      nc.vector.scalar_tensor_tensor(
                          ob[:, jc * 512:(jc + 1) * 512],
                          ops[:],
                          1.0,
                          bo_bc[:, jc * 512:(jc + 1) * 512],
                          op0=mybir.AluOpType.mult,
                          op1=mybir.AluOpType.add,
                      )
                      nc.sync.dma_start(
                          out_e[rb * 128:(rb + 1) * 128,
                                jc * 512:(jc + 1) * 512],
                          ob[:, jc * 512:(jc + 1) * 512],
                      )
        per.release()
        cst.release()

    nc.finalize()
    return nc


_CACHE = {}


def _get_nc(with_padding: bool):
    if with_padding not in _CACHE:
        _CACHE[with_padding] = _build(with_padding)
    return _CACHE[with_padding]


def _prepare_in_maps(x, Wq, Wk, Wv, Wo, bo, key_padding_mask):
    bf = ml_dtypes.bfloat16
    x = np.asarray(x, dtype=np.float32)
    Wq = np.asarray(Wq, dtype=np.float32)
    Wk = np.asarray(Wk, dtype=np.float32)
    Wv = np.asarray(Wv, dtype=np.float32)
    Wo = np.asarray(Wo, dtype=np.float32)
    bo = np.asarray(bo, dtype=np.float32)
    pad = np.asarray(key_padding_mask)

    with_padding = bool(pad.any())

    # [g, dc, p, t]: one 128KB DMA per (chunk, d-chunk).
    xT = np.ascontiguousarray(
        x.reshape(B * NQC, QC, DC, 128).transpose(0, 2, 3, 1)
    ).astype(bf)
    # Fold the 1/sqrt(head_dim) score scale into Wq (power of two: exact).
    Wq_s = Wq * np.float32(1.0 / np.sqrt(HD))

    # tri[k, c] = 1.0 where k <= c (keep), 0 above-diagonal k > c.
    tri = (np.arange(128)[:, None] <= np.arange(128)[None, :]).astype(bf)
    ident = np.eye(128, dtype=np.float32)
    ones64 = np.ones((1, HD), dtype=np.float32)
    bo_row = np.ascontiguousarray(bo.reshape(1, D))

    def wsb(W, cols):
        # SBUF layout [128, DC*128]: [p, dc*128 + c] = W[dc*128 + p, cols[c]]
        blk = W[:, cols].reshape(DC, 128, 128)
        return np.ascontiguousarray(
            blk.transpose(1, 0, 2).reshape(128, DC * 128)
        ).astype(bf)

    wo3 = np.ascontiguousarray(
        Wo.reshape(DC, 128, D).transpose(1, 0, 2).reshape(128, DC * D)
    ).astype(bf)

    in_maps = []
    for c in range(N_CORES):
        cols = slice(c * 128, (c + 1) * 128)
        m = {
            "xT": xT,
            "wq": wsb(Wq_s, cols),
            "wk": wsb(Wk, cols),
            "wv": wsb(Wv, cols),
            "wo": wo3,
            "bo_row": bo_row,
            "tri": tri,
            "ident": ident,
            "ones64": ones64,
        }
        if with_padding:
            m["padcol"] = np.ascontiguousarray(
                (~pad).astype(np.float32).reshape(B, NTB, 128, 1)
            )
        in_maps.append(m)
    return with_padding, in_maps


def _run(with_padding, in_maps, trace=False):
    nc = _get_nc(with_padding)
    return run_bass_kernel_spmd(
        nc, in_maps, core_ids=list(range(N_CORES)), trace=trace
    )


def kernel(x, Wq, Wk, Wv, Wo, bo, key_padding_mask):
    with_padding, in_maps = _prepare_in_maps(
        x, Wq, Wk, Wv, Wo, bo, key_padding_mask
    )
    res = _run(with_padding, in_maps)
    # Core c's out row-block rb=2b+qh covers rows [qh*1024 + c*128, +128).
    full = np.empty((B, T, D), dtype=np.float32)
    for c in range(N_CORES):
        o = res.results[c]["out"]
        for b in range(B):
            for qh in range(2):
                r0 = qh * 1024 + c * 128
                full[b, r0:r0 + 128] = o[(2 * b + qh) * 128:
                                         (2 * b + qh + 1) * 128]
    return full
